# revision 17
# baseline (speedup 1.0000x reference)
"""Bass/Trainium2 kernel for nn_DimeNet_22737556865501 (optimized v2).

Same math as the baseline (circulant-graph collapse to dense per-atom work)
with these performance changes:

1. delta-symmetry: alpha[a,b] is symmetric, so the pair chain (ab, amg, den,
   ratio, arctan) is computed only for the 8 circular shifts delta=1..8
   ([K, delta, a] layout, a innermost/unit-stride) instead of all 256 (a,b)
   pairs; rows delta=9..15 of the contraction tensor are cheap copies
   (alpha[d', a] = alpha[16-d', (a+d') % 16]).
2. ratio = sqrt(amg/den) is computed as amg * rsqrt(|amg*den|) using the
   single ACT function Abs_reciprocal_sqrt, so pass 1 needs ONE activation
   table set (abs_reciprocal_sqrt_and_small) and one transcendental op per
   pair instead of five (Square/Ln/Square/Ln/Exp).  1/d and d also come from
   the same function.  Clamps (max(amg,0), max(prod,1e-30)) make it NaN-free.
3. bf16 + DVE 2x mode for everything after the cancellation-sensitive
   subtract amg = ab - G (which stays f32 internally, bf16 output): the
   clamp/prod/ratio chain, the full contraction (products + binary tree),
   all with unit-stride innermost APs.  Shifted reads (a+delta) are split
   into even-delta (4B-aligned -> 2x) and odd-delta instructions.
4. Engine rebalance: GPSIMD runs the odd-delta contraction products and the
   sin-argument ops; ACT runs the transcendentals, the dc-power squares and
   scaling; DVE the rest.

Sharding: unchanged — atoms partitioned across 8 cores, no collective.
"""

import numpy as np

N_ATOMS = 32768
DEG = 16
HALF = DEG // 2
N_CORES = 8
J_PER_CORE = N_ATOMS // N_CORES  # 4096
P = 128
N_TILES = J_PER_CORE // P  # 32
K_BATCH = 8
N_SUPER = N_TILES // K_BATCH  # 4
WIN_ROWS = J_PER_CORE + DEG  # 4112
N_RBF = 6
CUTOFF = 5.0
ENV_P = 6
A_ = -(ENV_P + 1) * (ENV_P + 2) / 2.0  # -28
B_ = float(ENV_P * (ENV_P + 2))  # 48
C_ = -ENV_P * (ENV_P + 1) / 2.0  # -21
TWO_PI = float(2.0 * np.pi)
INV_2PI = float(1.0 / (2.0 * np.pi))

OFFS = np.concatenate([np.arange(1, HALF + 1), -np.arange(1, HALF + 1)])

_cached_nc = None


def _expected_graph():
    half = HALF
    offs = np.concatenate([np.arange(1, half + 1), N_ATOMS - np.arange(1, half + 1)])
    j = np.arange(N_ATOMS)
    nbr_dst = (j[:, None] + offs[None, :]) % N_ATOMS
    nbr_list = np.stack([np.repeat(j, DEG), nbr_dst.reshape(-1)], 1)
    o1, o2 = np.meshgrid(offs, offs, indexing="ij")
    keep = o1 != o2
    o1, o2 = o1[keep], o2[keep]
    i = (j[:, None] + o1[None, :]) % N_ATOMS
    k = (j[:, None] + o2[None, :]) % N_ATOMS
    jc = np.broadcast_to(j[:, None], i.shape)
    angle_list = np.stack([i.reshape(-1), jc.reshape(-1), k.reshape(-1)], 1)
    return nbr_list.astype(np.int64), angle_list.astype(np.int64)


def _graph_matches(nbr_list, angle_list):
    if nbr_list.shape != (N_ATOMS * DEG, 2):
        return False
    if angle_list.shape != (N_ATOMS * DEG * (DEG - 1), 3):
        return False
    exp_nbr, exp_ang = _expected_graph()
    return np.array_equal(np.asarray(nbr_list), exp_nbr) and np.array_equal(
        np.asarray(angle_list), exp_ang
    )


def _fallback_numpy(xyz, nbr_list, angle_list):
    """Exact numpy mirror of the jax reference (general graph)."""
    xyz = np.asarray(xyz, dtype=np.float32)
    nbr = np.asarray(nbr_list)
    ang = np.asarray(angle_list)
    E = nbr.shape[0]
    r_ji = xyz[ang[:, 0]] - xyz[ang[:, 1]]
    r_jk = xyz[ang[:, 2]] - xyz[ang[:, 1]]
    dot = np.sum(r_ji * r_jk, axis=-1)
    crs = np.linalg.norm(np.cross(r_ji, r_jk), axis=-1)
    alpha = np.arctan2(crs, dot)
    diff = xyz[nbr[:, 0]] - xyz[nbr[:, 1]]
    d = np.linalg.norm(diff, axis=-1)
    n = np.arange(1, N_RBF + 1, dtype=xyz.dtype)
    dc = (d / CUTOFF)[:, None]
    env = 1.0 / dc + A_ * dc ** (ENV_P - 1) + B_ * dc**ENV_P + C_ * dc ** (ENV_P + 1)
    e_rbf = env * np.sin(n * np.pi * dc)
    keys = nbr[:, 0] * N_ATOMS + nbr[:, 1]
    order = np.argsort(keys, kind="stable")
    ji_idx = order[np.searchsorted(keys[order], ang[:, 1] * N_ATOMS + ang[:, 0])]
    kj_idx = order[np.searchsorted(keys[order], ang[:, 2] * N_ATOMS + ang[:, 1])]
    trip = alpha[:, None] * e_rbf[kj_idx]
    out = np.zeros((E, N_RBF), dtype=np.float32)
    np.add.at(out, ji_idx, trip.astype(np.float32))
    return out


# ---------------------------------------------------------------------------
# Device kernel
# ---------------------------------------------------------------------------


def _build_device_kernel():
    import concourse.bacc as bacc
    import concourse.mybir as mybir
    from concourse.bass_types import AP
    from concourse.tile import TileContext

    F32 = mybir.dt.float32
    BF16 = mybir.dt.bfloat16
    I32 = mybir.dt.int32
    ALU = mybir.AluOpType
    ACT = mybir.ActivationFunctionType

    # Steer activation-table loading: pass 1 resolves only to
    # abs_reciprocal_sqrt_and_small, pass 2 only to trig_and_small, so the
    # whole kernel performs exactly two table loads.
    from concourse.hw_specs import get_activation_tables

    tabs = get_activation_tables("gen3")
    for name, fns in tabs.items():
        if name != "abs_reciprocal_sqrt_and_small":
            fns.discard(ACT.Abs_reciprocal_sqrt)
        if name != "trig_and_small":
            fns.discard(ACT.Sin)
            fns.discard(ACT.Arctan)
        if name not in ("abs_reciprocal_sqrt_and_small", "trig_and_small"):
            fns.discard(ACT.Copy)
            fns.discard(ACT.Identity)
            fns.discard(ACT.Square)

    def sub(base: AP, off: int, dims) -> AP:
        return AP(
            tensor=base.tensor,
            offset=base.offset + off,
            ap=[list(base.ap[0]), *[list(d) for d in dims]],
        )

    nc = bacc.Bacc("TRN2", target_bir_lowering=False, debug=False, num_devices=N_CORES)
    win = nc.dram_tensor("win", [WIN_ROWS, 3], F32, kind="ExternalInput")
    consts = nc.dram_tensor("consts", [P, 16], F32, kind="ExternalInput")
    out = nc.dram_tensor("out", [J_PER_CORE * DEG, N_RBF], F32, kind="ExternalOutput")

    K = K_BATCH
    with TileContext(nc) as tc:
        with (
            tc.tile_pool(name="cst", bufs=1) as cpool,
            tc.tile_pool(name="carry", bufs=1) as carry,
            tc.tile_pool(name="p1", bufs=2) as p1,
            tc.tile_pool(name="p2", bufs=1) as p2,
        ):
            nco = cpool.tile([P, 16], F32)
            nc.sync.dma_start(nco[:], consts[:])

            ratio_t = [carry.tile([P, K * 128], BF16, name=f"ratio{s}") for s in range(N_SUPER)]
            sred_t = [carry.tile([P, K * 96], F32, name=f"sred{s}") for s in range(N_SUPER)]
            env_t = [carry.tile([P, K * 16], F32, name=f"env{s}") for s in range(N_SUPER)]

            # ---------------- pass 1: geometry + arsqrt ----------------
            for s in range(N_SUPER):
                w = p1.tile([P, K * 51], F32, tag="w")
                vt = p1.tile([P, K * 72], F32, tag="vt")  # [K][c3][a24]
                g = p1.tile([P, K * 144], F32, tag="g")  # [K][d9][a16]
                gtmp = p1.tile([P, K * 144], F32, tag="gtmp")
                d24 = p1.tile([P, K * 24], F32, tag="d24")
                arsq0 = p1.tile([P, K * 16], F32, tag="arsq0")
                ab = p1.tile([P, K * 128], F32, tag="ab")  # [K][d8][a16]
                amg = p1.tile([P, K * 128], BF16, tag="amg")
                den = p1.tile([P, K * 128], BF16, tag="den")
                amgc = p1.tile([P, K * 128], BF16, tag="amgc")
                prod = p1.tile([P, K * 128], BF16, tag="prod")
                prodc = p1.tile([P, K * 128], BF16, tag="prodc")
                r2 = p1.tile([P, K * 128], BF16, tag="r2")
                dcp = p1.tile([P, K * 16], F32, tag="dcp")
                inv2 = p1.tile([P, K * 16], F32, tag="inv2")
                dc2 = p1.tile([P, K * 16], F32, tag="dc2")
                dc4 = p1.tile([P, K * 16], F32, tag="dc4")
                x5 = p1.tile([P, K * 16], F32, tag="x5")
                h1 = p1.tile([P, K * 16], F32, tag="h1")
                h2 = p1.tile([P, K * 16], F32, tag="h2")
                h3 = p1.tile([P, K * 16], F32, tag="h3")
                sa2 = p1.tile([P, K * 96], F32, tag="sa2")  # [K][n6][b16]
                st2 = p1.tile([P, K * 96], F32, tag="st2")
                ki = p1.tile([P, K * 96], I32, tag="ki")
                kf = p1.tile([P, K * 96], F32, tag="kf")

                src = AP(tensor=win, offset=s * K * P * 3, ap=[[3, P], [P * 3, K], [1, 51]])
                nc.sync.dma_start(w[:], src)
                wa, va = w[:], vt[:]

                # V_T[K][c][a]: a=0..7 <- +1..+8 (win rows +9..+16), a=8..15 <- -1..-8
                nc.vector.tensor_tensor(
                    sub(va, 0, [[72, K], [24, 3], [1, 8]]),
                    sub(wa, 27, [[51, K], [1, 3], [3, 8]]),
                    sub(wa, 24, [[51, K], [1, 3], [0, 8]]),
                    ALU.subtract,
                )
                nc.vector.tensor_tensor(
                    sub(va, 8, [[72, K], [24, 3], [1, 8]]),
                    sub(wa, 21, [[51, K], [1, 3], [-3, 8]]),
                    sub(wa, 24, [[51, K], [1, 3], [0, 8]]),
                    ALU.subtract,
                )
                # circular halo: V_T[c][16+y] = V_T[c][y]
                nc.vector.tensor_copy(
                    sub(va, 16, [[72, K], [24, 3], [1, 8]]),
                    sub(va, 0, [[72, K], [24, 3], [1, 8]]),
                )

                # Gram, delta-packed: G[d][a] = sum_c V[a+d]c * V[a]c, d=0..8
                ga = g[:]
                nc.gpsimd.tensor_tensor(
                    sub(ga, 0, [[144, K], [16, 9], [1, 16]]),
                    sub(va, 0, [[72, K], [1, 9], [1, 16]]),
                    sub(va, 0, [[72, K], [0, 9], [1, 16]]),
                    ALU.mult,
                )
                for c, gt in ((1, gtmp), (2, gtmp)):
                    nc.gpsimd.tensor_tensor(
                        sub(gt[:], 0, [[144, K], [16, 9], [1, 16]]),
                        sub(va, 24 * c, [[72, K], [1, 9], [1, 16]]),
                        sub(va, 24 * c, [[72, K], [0, 9], [1, 16]]),
                        ALU.mult,
                    )
                    nc.vector.tensor_tensor(
                        sub(ga, 0, [[144, K], [1, 144]]),
                        sub(ga, 0, [[144, K], [1, 144]]),
                        sub(gt[:], 0, [[144, K], [1, 144]]),
                        ALU.add,
                    )

                # 1/d and d from row delta=0 (n2), plus circular halo of d
                nc.scalar.activation(
                    sub(arsq0[:], 0, [[16, K], [1, 16]]),
                    sub(ga, 0, [[144, K], [1, 16]]),
                    ACT.Abs_reciprocal_sqrt,
                )
                nc.vector.tensor_tensor(
                    sub(d24[:], 0, [[24, K], [1, 16]]),
                    sub(ga, 0, [[144, K], [1, 16]]),
                    sub(arsq0[:], 0, [[16, K], [1, 16]]),
                    ALU.mult,
                )
                nc.vector.tensor_copy(
                    sub(d24[:], 16, [[24, K], [1, 8]]),
                    sub(d24[:], 0, [[24, K], [1, 8]]),
                )

                # ab[d][a] = d[a] * d[a+d], d=1..8 (odd/even split by alignment)
                nc.vector.tensor_tensor(
                    sub(ab[:], 0, [[128, K], [32, 4], [1, 16]]),
                    sub(d24[:], 1, [[24, K], [2, 4], [1, 16]]),
                    sub(d24[:], 0, [[24, K], [0, 4], [1, 16]]),
                    ALU.mult,
                )
                nc.vector.tensor_tensor(
                    sub(ab[:], 16, [[128, K], [32, 4], [1, 16]]),
                    sub(d24[:], 2, [[24, K], [2, 4], [1, 16]]),
                    sub(d24[:], 0, [[24, K], [0, 4], [1, 16]]),
                    ALU.mult,
                )

                # amg/den (f32 compute, bf16 store), clamp, prod, ratio
                gp = sub(ga, 16, [[144, K], [16, 8], [1, 16]])
                abf = sub(ab[:], 0, [[128, K], [16, 8], [1, 16]])
                nc.vector.tensor_tensor(
                    sub(amg[:], 0, [[128, K], [16, 8], [1, 16]]), abf, gp, ALU.subtract
                )
                nc.vector.tensor_tensor(
                    sub(den[:], 0, [[128, K], [16, 8], [1, 16]]), abf, gp, ALU.add
                )
                nc.vector.tensor_scalar_max(amgc[:], amg[:], 0.0)
                nc.vector.tensor_tensor(prod[:], amgc[:], den[:], ALU.mult)
                nc.vector.tensor_scalar_max(prodc[:], prod[:], 1e-30)
                nc.scalar.activation(r2[:], prodc[:], ACT.Abs_reciprocal_sqrt)
                nc.vector.tensor_tensor(ratio_t[s][:], amgc[:], r2[:], ALU.mult)

                # envelope: env2 = 2C'/d*? -> inv2 + dc^5*(2A + dc*(2B + dc*2C'))
                nc.scalar.mul(
                    sub(dcp[:], 0, [[16, K], [1, 16]]),
                    sub(d24[:], 0, [[24, K], [1, 16]]),
                    1.0 / CUTOFF,
                )
                nc.scalar.mul(inv2[:], arsq0[:], 2.0 * CUTOFF)
                nc.vector.tensor_tensor(dc2[:], dcp[:], dcp[:], ALU.mult)
                nc.vector.tensor_tensor(dc4[:], dc2[:], dc2[:], ALU.mult)
                nc.vector.tensor_tensor(x5[:], dc4[:], dcp[:], ALU.mult)
                nc.vector.tensor_scalar(h1[:], dcp[:], 2.0 * C_, 2.0 * B_, ALU.mult, ALU.add)
                nc.vector.tensor_tensor(h2[:], h1[:], dcp[:], ALU.mult)
                nc.vector.scalar_tensor_tensor(h3[:], h2[:], 2.0 * A_, x5[:], ALU.add, ALU.mult)
                nc.vector.tensor_tensor(env_t[s][:], h3[:], inv2[:], ALU.add)

                # sin args [K][n6][b16], range-reduced to [-pi, pi]
                nc.gpsimd.tensor_tensor(
                    sub(sa2[:], 0, [[96, K], [16, 6], [1, 16]]),
                    sub(dcp[:], 0, [[16, K], [0, 6], [1, 16]]),
                    sub(nco[:], 0, [[0, K], [1, 6], [0, 16]]),
                    ALU.mult,
                )
                nc.scalar.mul(st2[:], sa2[:], INV_2PI)
                nc.vector.tensor_copy(ki[:], st2[:])  # round-to-nearest
                nc.vector.tensor_copy(kf[:], ki[:])
                nc.vector.scalar_tensor_tensor(
                    sred_t[s][:], kf[:], -TWO_PI, sa2[:], ALU.mult, ALU.add
                )

            # ---------------- pass 2: trig + contraction ----------------
            tc.no_sync_barrier()
            for s in range(N_SUPER):
                aext = p2.tile([P, K * 256], F32, tag="aext")  # [K][d16][a16]
                sinv = p2.tile([P, K * 96], F32, tag="sinv")  # [K][r6][b16]
                erbf = p2.tile([P, K * 192], F32, tag="erbf")  # [K][r6][b32]
                v16 = p2.tile([P, K * 1536], F32, tag="v16")  # [K][r6][d16][a16]
                ot = p2.tile([P, K * 96], F32, tag="ot")  # [K][a16][r6]

                ax = aext[:]
                nc.scalar.activation(
                    sub(ax, 16, [[256, K], [1, 128]]),
                    sub(ratio_t[s][:], 0, [[128, K], [1, 128]]),
                    ACT.Arctan,
                )
                nc.gpsimd.memset(sub(ax, 0, [[256, K], [1, 16]]), 0.0)
                # duplicate rows 9..15: aext[d'][a] = aext[16-d'][(a+d')%16]
                for dp in range(9, 16):
                    sb = (16 - dp) * 16
                    nc.scalar.copy(
                        sub(ax, dp * 16, [[256, K], [1, 16 - dp]]),
                        sub(ax, sb + dp, [[256, K], [1, 16 - dp]]),
                    )
                    nc.scalar.copy(
                        sub(ax, dp * 16 + (16 - dp), [[256, K], [1, dp]]),
                        sub(ax, sb, [[256, K], [1, dp]]),
                    )

                nc.scalar.activation(sinv[:], sred_t[s][:], ACT.Sin)
                # erbf[r][b] = env2[b] * sin_r[b], plus circular halo b=16..31
                eb = erbf[:]
                nc.vector.tensor_tensor(
                    sub(eb, 0, [[192, K], [32, 6], [1, 16]]),
                    sub(sinv[:], 0, [[96, K], [16, 6], [1, 16]]),
                    sub(env_t[s][:], 0, [[16, K], [0, 6], [1, 16]]),
                    ALU.mult,
                )
                nc.vector.tensor_copy(
                    sub(eb, 16, [[192, K], [32, 6], [1, 16]]),
                    sub(eb, 0, [[192, K], [32, 6], [1, 16]]),
                )

                # products v16[r][d][a] = aext[d][a] * erbf[r][a+d]
                # out/in0 are flat contiguous 256-runs; in1 is a sliding
                # 16-window (d and a both stride 1)
                vv = v16[:]
                for r in range(N_RBF):
                    eng = nc.vector if r < 4 else nc.gpsimd
                    eng.tensor_tensor(
                        sub(vv, r * 256, [[1536, K], [1, 256]]),
                        sub(ax, 0, [[256, K], [1, 256]]),
                        sub(eb, r * 32, [[192, K], [1, 16], [1, 16]]),
                        ALU.mult,
                    )
                # binary tree over d (in place), then f32 transpose-store to ot
                for half in (128, 64, 32):
                    nc.vector.tensor_tensor(
                        sub(vv, 0, [[1536, K], [256, 6], [1, half]]),
                        sub(vv, 0, [[1536, K], [256, 6], [1, half]]),
                        sub(vv, half, [[1536, K], [256, 6], [1, half]]),
                        ALU.add,
                    )
                nc.vector.tensor_tensor(
                    sub(ot[:], 0, [[96, K], [1, 6], [6, 16]]),
                    sub(vv, 0, [[1536, K], [256, 6], [1, 16]]),
                    sub(vv, 16, [[1536, K], [256, 6], [1, 16]]),
                    ALU.add,
                )
                dst = AP(
                    tensor=out,
                    offset=s * K * P * 96,
                    ap=[[96, P], [96 * P, K], [1, 96]],
                )
                nc.sync.dma_start(dst, ot[:])

    nc.compile()
    return nc


def _get_nc():
    global _cached_nc
    if _cached_nc is None:
        _cached_nc = _build_device_kernel()
    return _cached_nc


def _make_consts():
    ncv = np.zeros(16, np.float32)
    ncv[:N_RBF] = (np.arange(1, N_RBF + 1) * np.pi).astype(np.float32)
    return np.broadcast_to(ncv, (P, 16)).copy()


def _run_device(xyz, trace=False, tmpdir=None):
    from concourse import bass_utils

    nc = _get_nc()
    consts = _make_consts()
    ext = np.concatenate([xyz[-HALF:], xyz, xyz[:HALF]], axis=0)
    in_maps = []
    for c in range(N_CORES):
        base = c * J_PER_CORE
        winc = np.ascontiguousarray(ext[base : base + WIN_ROWS])
        in_maps.append({"win": winc, "consts": consts})
    kwargs = {}
    if trace:
        kwargs = dict(trace=True)
        if tmpdir is not None:
            kwargs["tmpdir"] = tmpdir
    res = bass_utils.run_bass_kernel_spmd(
        nc, in_maps, core_ids=list(range(N_CORES)), **kwargs
    )
    shards = [res.results[c]["out"] for c in range(N_CORES)]
    full = np.concatenate(shards, axis=0).astype(np.float32)
    return full, res


def kernel(xyz, nbr_list, angle_list):
    xyz = np.asarray(xyz, dtype=np.float32)
    if not _graph_matches(nbr_list, angle_list):
        return _fallback_numpy(xyz, nbr_list, angle_list)
    out, _ = _run_device(xyz)
    return out


# revision 18
# speedup vs baseline: 1.1252x; 1.1252x over previous
"""Bass/Trainium2 kernel for nn_DimeNet_22737556865501 (optimized v2).

Same math as the baseline (circulant-graph collapse to dense per-atom work)
with these performance changes:

1. delta-symmetry: alpha[a,b] is symmetric, so the pair chain (ab, amg, den,
   ratio, arctan) is computed only for the 8 circular shifts delta=1..8
   ([K, delta, a] layout, a innermost/unit-stride) instead of all 256 (a,b)
   pairs; rows delta=9..15 of the contraction tensor are cheap copies
   (alpha[d', a] = alpha[16-d', (a+d') % 16]).
2. ratio = sqrt(amg/den) is computed as amg * rsqrt(|amg*den|) using the
   single ACT function Abs_reciprocal_sqrt, so pass 1 needs ONE activation
   table set (abs_reciprocal_sqrt_and_small) and one transcendental op per
   pair instead of five (Square/Ln/Square/Ln/Exp).  1/d and d also come from
   the same function.  Clamps (max(amg,0), max(prod,1e-30)) make it NaN-free.
3. bf16 + DVE 2x mode for everything after the cancellation-sensitive
   subtract amg = ab - G (which stays f32 internally, bf16 output): the
   clamp/prod/ratio chain, the full contraction (products + binary tree),
   all with unit-stride innermost APs.  Shifted reads (a+delta) are split
   into even-delta (4B-aligned -> 2x) and odd-delta instructions.
4. Engine rebalance: GPSIMD runs the odd-delta contraction products and the
   sin-argument ops; ACT runs the transcendentals, the dc-power squares and
   scaling; DVE the rest.

Sharding: unchanged — atoms partitioned across 8 cores, no collective.
"""

import numpy as np

N_ATOMS = 32768
DEG = 16
HALF = DEG // 2
N_CORES = 8
J_PER_CORE = N_ATOMS // N_CORES  # 4096
P = 128
N_TILES = J_PER_CORE // P  # 32
K_BATCH = 8
N_SUPER = N_TILES // K_BATCH  # 4
WIN_ROWS = J_PER_CORE + DEG  # 4112
N_RBF = 6
CUTOFF = 5.0
ENV_P = 6
A_ = -(ENV_P + 1) * (ENV_P + 2) / 2.0  # -28
B_ = float(ENV_P * (ENV_P + 2))  # 48
C_ = -ENV_P * (ENV_P + 1) / 2.0  # -21
TWO_PI = float(2.0 * np.pi)
INV_2PI = float(1.0 / (2.0 * np.pi))

OFFS = np.concatenate([np.arange(1, HALF + 1), -np.arange(1, HALF + 1)])

_cached_nc = None


def _expected_graph():
    half = HALF
    offs = np.concatenate([np.arange(1, half + 1), N_ATOMS - np.arange(1, half + 1)])
    j = np.arange(N_ATOMS)
    nbr_dst = (j[:, None] + offs[None, :]) % N_ATOMS
    nbr_list = np.stack([np.repeat(j, DEG), nbr_dst.reshape(-1)], 1)
    o1, o2 = np.meshgrid(offs, offs, indexing="ij")
    keep = o1 != o2
    o1, o2 = o1[keep], o2[keep]
    i = (j[:, None] + o1[None, :]) % N_ATOMS
    k = (j[:, None] + o2[None, :]) % N_ATOMS
    jc = np.broadcast_to(j[:, None], i.shape)
    angle_list = np.stack([i.reshape(-1), jc.reshape(-1), k.reshape(-1)], 1)
    return nbr_list.astype(np.int64), angle_list.astype(np.int64)


def _graph_matches(nbr_list, angle_list):
    if nbr_list.shape != (N_ATOMS * DEG, 2):
        return False
    if angle_list.shape != (N_ATOMS * DEG * (DEG - 1), 3):
        return False
    exp_nbr, exp_ang = _expected_graph()
    return np.array_equal(np.asarray(nbr_list), exp_nbr) and np.array_equal(
        np.asarray(angle_list), exp_ang
    )


def _fallback_numpy(xyz, nbr_list, angle_list):
    """Exact numpy mirror of the jax reference (general graph)."""
    xyz = np.asarray(xyz, dtype=np.float32)
    nbr = np.asarray(nbr_list)
    ang = np.asarray(angle_list)
    E = nbr.shape[0]
    r_ji = xyz[ang[:, 0]] - xyz[ang[:, 1]]
    r_jk = xyz[ang[:, 2]] - xyz[ang[:, 1]]
    dot = np.sum(r_ji * r_jk, axis=-1)
    crs = np.linalg.norm(np.cross(r_ji, r_jk), axis=-1)
    alpha = np.arctan2(crs, dot)
    diff = xyz[nbr[:, 0]] - xyz[nbr[:, 1]]
    d = np.linalg.norm(diff, axis=-1)
    n = np.arange(1, N_RBF + 1, dtype=xyz.dtype)
    dc = (d / CUTOFF)[:, None]
    env = 1.0 / dc + A_ * dc ** (ENV_P - 1) + B_ * dc**ENV_P + C_ * dc ** (ENV_P + 1)
    e_rbf = env * np.sin(n * np.pi * dc)
    keys = nbr[:, 0] * N_ATOMS + nbr[:, 1]
    order = np.argsort(keys, kind="stable")
    ji_idx = order[np.searchsorted(keys[order], ang[:, 1] * N_ATOMS + ang[:, 0])]
    kj_idx = order[np.searchsorted(keys[order], ang[:, 2] * N_ATOMS + ang[:, 1])]
    trip = alpha[:, None] * e_rbf[kj_idx]
    out = np.zeros((E, N_RBF), dtype=np.float32)
    np.add.at(out, ji_idx, trip.astype(np.float32))
    return out


# ---------------------------------------------------------------------------
# Device kernel
# ---------------------------------------------------------------------------


def _build_device_kernel():
    import concourse.bacc as bacc
    import concourse.mybir as mybir
    from concourse.bass_types import AP
    from concourse.tile import TileContext

    F32 = mybir.dt.float32
    BF16 = mybir.dt.bfloat16
    I32 = mybir.dt.int32
    ALU = mybir.AluOpType
    ACT = mybir.ActivationFunctionType

    # Steer activation-table loading: pass 1 resolves only to
    # abs_reciprocal_sqrt_and_small, pass 2 only to trig_and_small, so the
    # whole kernel performs exactly two table loads.
    from concourse.hw_specs import get_activation_tables

    tabs = get_activation_tables("gen3")
    for name, fns in tabs.items():
        if name != "abs_reciprocal_sqrt_and_small":
            fns.discard(ACT.Abs_reciprocal_sqrt)
        if name != "trig_and_small":
            fns.discard(ACT.Sin)
            fns.discard(ACT.Arctan)
        if name not in ("abs_reciprocal_sqrt_and_small", "trig_and_small"):
            fns.discard(ACT.Copy)
            fns.discard(ACT.Identity)
            fns.discard(ACT.Square)

    def sub(base: AP, off: int, dims) -> AP:
        return AP(
            tensor=base.tensor,
            offset=base.offset + off,
            ap=[list(base.ap[0]), *[list(d) for d in dims]],
        )

    nc = bacc.Bacc("TRN2", target_bir_lowering=False, debug=False, num_devices=N_CORES)
    win = nc.dram_tensor("win", [WIN_ROWS, 3], F32, kind="ExternalInput")
    consts = nc.dram_tensor("consts", [P, 16], F32, kind="ExternalInput")
    out = nc.dram_tensor("out", [J_PER_CORE * DEG, N_RBF], F32, kind="ExternalOutput")

    K = K_BATCH
    with TileContext(nc) as tc:
        with (
            tc.tile_pool(name="cst", bufs=1) as cpool,
            tc.tile_pool(name="carry", bufs=1) as carry,
            tc.tile_pool(name="p1", bufs=2) as p1,
            tc.tile_pool(name="p2", bufs=1) as p2,
        ):
            nco = cpool.tile([P, 16], F32)
            nc.sync.dma_start(nco[:], consts[:])

            ratio_t = [carry.tile([P, K * 128], BF16, name=f"ratio{s}") for s in range(N_SUPER)]
            sred_t = [carry.tile([P, K * 96], F32, name=f"sred{s}") for s in range(N_SUPER)]
            env_t = [carry.tile([P, K * 16], BF16, name=f"env{s}") for s in range(N_SUPER)]

            # ---------------- pass 1: geometry + arsqrt ----------------
            for s in range(N_SUPER):
                w = p1.tile([P, K * 51], F32, tag="w")
                vt = p1.tile([P, K * 72], F32, tag="vt")  # [K][c3][a24]
                g = p1.tile([P, K * 144], F32, tag="g")  # [K][d9][a16]
                gtmp = p1.tile([P, K * 144], F32, tag="gtmp")
                gtmp2 = p1.tile([P, K * 144], F32, tag="gtmp2")
                d24 = p1.tile([P, K * 24], F32, tag="d24")
                arsq0 = p1.tile([P, K * 16], F32, tag="arsq0")
                ab = p1.tile([P, K * 128], F32, tag="ab")  # [K][d8][a16]
                amg = p1.tile([P, K * 128], BF16, tag="amg")
                den = p1.tile([P, K * 128], BF16, tag="den")
                amgc = p1.tile([P, K * 128], BF16, tag="amgc")
                prod = p1.tile([P, K * 128], BF16, tag="prod")
                prodc = p1.tile([P, K * 128], BF16, tag="prodc")
                r2 = p1.tile([P, K * 128], BF16, tag="r2")
                dcp = p1.tile([P, K * 16], F32, tag="dcp")
                inv2 = p1.tile([P, K * 16], F32, tag="inv2")
                dc2 = p1.tile([P, K * 16], F32, tag="dc2")
                dc4 = p1.tile([P, K * 16], F32, tag="dc4")
                x5 = p1.tile([P, K * 16], F32, tag="x5")
                h1 = p1.tile([P, K * 16], F32, tag="h1")
                h2 = p1.tile([P, K * 16], F32, tag="h2")
                h3 = p1.tile([P, K * 16], F32, tag="h3")
                sa2 = p1.tile([P, K * 96], F32, tag="sa2")  # [K][n6][b16]
                st2 = p1.tile([P, K * 96], F32, tag="st2")
                ki = p1.tile([P, K * 96], I32, tag="ki")
                kf = p1.tile([P, K * 96], F32, tag="kf")

                src = AP(tensor=win, offset=s * K * P * 3, ap=[[3, P], [P * 3, K], [1, 51]])
                nc.sync.dma_start(w[:], src)
                wa, va = w[:], vt[:]

                # V_T[K][c][a]: a=0..7 <- +1..+8 (win rows +9..+16), a=8..15 <- -1..-8
                nc.vector.tensor_tensor(
                    sub(va, 0, [[72, K], [24, 3], [1, 8]]),
                    sub(wa, 27, [[51, K], [1, 3], [3, 8]]),
                    sub(wa, 24, [[51, K], [1, 3], [0, 8]]),
                    ALU.subtract,
                )
                nc.vector.tensor_tensor(
                    sub(va, 8, [[72, K], [24, 3], [1, 8]]),
                    sub(wa, 21, [[51, K], [1, 3], [-3, 8]]),
                    sub(wa, 24, [[51, K], [1, 3], [0, 8]]),
                    ALU.subtract,
                )
                # circular halo: V_T[c][16+y] = V_T[c][y]
                nc.vector.tensor_copy(
                    sub(va, 16, [[72, K], [24, 3], [1, 8]]),
                    sub(va, 0, [[72, K], [24, 3], [1, 8]]),
                )

                # Gram, delta-packed: G[d][a] = sum_c V[a+d]c * V[a]c, d=0..8
                ga = g[:]
                nc.gpsimd.tensor_tensor(
                    sub(ga, 0, [[144, K], [16, 9], [1, 16]]),
                    sub(va, 0, [[72, K], [1, 9], [1, 16]]),
                    sub(va, 0, [[72, K], [0, 9], [1, 16]]),
                    ALU.mult,
                )
                for c, gt in ((1, gtmp), (2, gtmp2)):
                    nc.gpsimd.tensor_tensor(
                        sub(gt[:], 0, [[144, K], [16, 9], [1, 16]]),
                        sub(va, 24 * c, [[72, K], [1, 9], [1, 16]]),
                        sub(va, 24 * c, [[72, K], [0, 9], [1, 16]]),
                        ALU.mult,
                    )
                    nc.vector.tensor_tensor(
                        sub(ga, 0, [[144, K], [1, 144]]),
                        sub(ga, 0, [[144, K], [1, 144]]),
                        sub(gt[:], 0, [[144, K], [1, 144]]),
                        ALU.add,
                    )

                # 1/d and d from row delta=0 (n2), plus circular halo of d
                nc.scalar.activation(
                    sub(arsq0[:], 0, [[16, K], [1, 16]]),
                    sub(ga, 0, [[144, K], [1, 16]]),
                    ACT.Abs_reciprocal_sqrt,
                )
                nc.vector.tensor_tensor(
                    sub(d24[:], 0, [[24, K], [1, 16]]),
                    sub(ga, 0, [[144, K], [1, 16]]),
                    sub(arsq0[:], 0, [[16, K], [1, 16]]),
                    ALU.mult,
                )
                nc.vector.tensor_copy(
                    sub(d24[:], 16, [[24, K], [1, 8]]),
                    sub(d24[:], 0, [[24, K], [1, 8]]),
                )

                # ab[d][a] = d[a] * d[a+d], d=1..8 (odd/even split by alignment)
                nc.vector.tensor_tensor(
                    sub(ab[:], 0, [[128, K], [32, 4], [1, 16]]),
                    sub(d24[:], 1, [[24, K], [2, 4], [1, 16]]),
                    sub(d24[:], 0, [[24, K], [0, 4], [1, 16]]),
                    ALU.mult,
                )
                nc.vector.tensor_tensor(
                    sub(ab[:], 16, [[128, K], [32, 4], [1, 16]]),
                    sub(d24[:], 2, [[24, K], [2, 4], [1, 16]]),
                    sub(d24[:], 0, [[24, K], [0, 4], [1, 16]]),
                    ALU.mult,
                )

                # amg/den (f32 compute, bf16 store), clamp, prod, ratio
                gp = sub(ga, 16, [[144, K], [16, 8], [1, 16]])
                abf = sub(ab[:], 0, [[128, K], [16, 8], [1, 16]])
                nc.vector.tensor_tensor(
                    sub(amg[:], 0, [[128, K], [16, 8], [1, 16]]), abf, gp, ALU.subtract
                )
                nc.vector.tensor_tensor(
                    sub(den[:], 0, [[128, K], [16, 8], [1, 16]]), abf, gp, ALU.add
                )
                nc.vector.tensor_scalar_max(amgc[:], amg[:], 0.0)
                nc.vector.tensor_tensor(prod[:], amgc[:], den[:], ALU.mult)
                nc.vector.tensor_scalar_max(prodc[:], prod[:], 1e-30)
                nc.scalar.activation(r2[:], prodc[:], ACT.Abs_reciprocal_sqrt)
                nc.vector.tensor_tensor(ratio_t[s][:], amgc[:], r2[:], ALU.mult)

                # envelope: env2 = 2C'/d*? -> inv2 + dc^5*(2A + dc*(2B + dc*2C'))
                nc.scalar.mul(
                    sub(dcp[:], 0, [[16, K], [1, 16]]),
                    sub(d24[:], 0, [[24, K], [1, 16]]),
                    1.0 / CUTOFF,
                )
                nc.scalar.mul(inv2[:], arsq0[:], 2.0 * CUTOFF)
                nc.vector.tensor_tensor(dc2[:], dcp[:], dcp[:], ALU.mult)
                nc.vector.tensor_tensor(dc4[:], dc2[:], dc2[:], ALU.mult)
                nc.vector.tensor_tensor(x5[:], dc4[:], dcp[:], ALU.mult)
                nc.vector.tensor_scalar(h1[:], dcp[:], 2.0 * C_, 2.0 * B_, ALU.mult, ALU.add)
                nc.vector.tensor_tensor(h2[:], h1[:], dcp[:], ALU.mult)
                nc.vector.scalar_tensor_tensor(h3[:], h2[:], 2.0 * A_, x5[:], ALU.add, ALU.mult)
                nc.vector.tensor_tensor(env_t[s][:], h3[:], inv2[:], ALU.add)

                # sin args [K][n6][b16], range-reduced to [-pi, pi]
                nc.gpsimd.tensor_tensor(
                    sub(sa2[:], 0, [[96, K], [16, 6], [1, 16]]),
                    sub(dcp[:], 0, [[16, K], [0, 6], [1, 16]]),
                    sub(nco[:], 0, [[0, K], [1, 6], [0, 16]]),
                    ALU.mult,
                )
                nc.scalar.mul(st2[:], sa2[:], INV_2PI)
                nc.vector.tensor_copy(ki[:], st2[:])  # round-to-nearest
                nc.vector.tensor_copy(kf[:], ki[:])
                nc.vector.scalar_tensor_tensor(
                    sred_t[s][:], kf[:], -TWO_PI, sa2[:], ALU.mult, ALU.add
                )

            # ---------------- pass 2: trig + contraction ----------------
            tc.no_sync_barrier()
            for s in range(N_SUPER):
                aext = p2.tile([P, K * 256], BF16, tag="aext")  # [K][d16][a16]
                sinv = p2.tile([P, K * 96], BF16, tag="sinv")  # [K][r6][b16]
                erbf = p2.tile([P, K * 192], F32, tag="erbf")  # [K][r6][b32]
                v16 = p2.tile([P, K * 1536], BF16, tag="v16")  # [K][r6][d16][a16]
                ot = p2.tile([P, K * 96], F32, tag="ot")  # [K][a16][r6]

                ax = aext[:]
                nc.scalar.activation(
                    sub(ax, 16, [[256, K], [1, 128]]),
                    sub(ratio_t[s][:], 0, [[128, K], [1, 128]]),
                    ACT.Arctan,
                )
                nc.gpsimd.memset(sub(ax, 0, [[256, K], [1, 16]]), 0.0)
                # duplicate rows 9..15: aext[d'][a] = aext[16-d'][(a+d')%16]
                for dp in range(9, 16):
                    sb = (16 - dp) * 16
                    nc.scalar.copy(
                        sub(ax, dp * 16, [[256, K], [1, 16 - dp]]),
                        sub(ax, sb + dp, [[256, K], [1, 16 - dp]]),
                    )
                    nc.scalar.copy(
                        sub(ax, dp * 16 + (16 - dp), [[256, K], [1, dp]]),
                        sub(ax, sb, [[256, K], [1, dp]]),
                    )

                nc.scalar.activation(sinv[:], sred_t[s][:], ACT.Sin)
                # erbf[r][b] = env2[b] * sin_r[b], plus circular halo b=16..31
                eb = erbf[:]
                nc.vector.tensor_tensor(
                    sub(eb, 0, [[192, K], [32, 6], [1, 16]]),
                    sub(sinv[:], 0, [[96, K], [16, 6], [1, 16]]),
                    sub(env_t[s][:], 0, [[16, K], [0, 6], [1, 16]]),
                    ALU.mult,
                )
                nc.vector.tensor_copy(
                    sub(eb, 16, [[192, K], [32, 6], [1, 16]]),
                    sub(eb, 0, [[192, K], [32, 6], [1, 16]]),
                )

                # products v16[r][d][a] = aext[d][a] * erbf[r][a+d]
                # out/in0 are flat contiguous 256-runs; in1 is a sliding
                # 16-window (d and a both stride 1)
                vv = v16[:]
                for r in range(N_RBF):
                    eng = nc.vector if r < 4 else nc.gpsimd
                    eng.tensor_tensor(
                        sub(vv, r * 256, [[1536, K], [1, 256]]),
                        sub(ax, 0, [[256, K], [1, 256]]),
                        sub(eb, r * 32, [[192, K], [1, 16], [1, 16]]),
                        ALU.mult,
                    )
                # binary tree over d (in place), then f32 transpose-store to ot
                for half in (128, 64, 32):
                    nc.vector.tensor_tensor(
                        sub(vv, 0, [[1536, K], [256, 6], [1, half]]),
                        sub(vv, 0, [[1536, K], [256, 6], [1, half]]),
                        sub(vv, half, [[1536, K], [256, 6], [1, half]]),
                        ALU.add,
                    )
                nc.vector.tensor_tensor(
                    sub(ot[:], 0, [[96, K], [1, 6], [6, 16]]),
                    sub(vv, 0, [[1536, K], [256, 6], [1, 16]]),
                    sub(vv, 16, [[1536, K], [256, 6], [1, 16]]),
                    ALU.add,
                )
                dst = AP(
                    tensor=out,
                    offset=s * K * P * 96,
                    ap=[[96, P], [96 * P, K], [1, 96]],
                )
                nc.sync.dma_start(dst, ot[:])

    nc.compile()
    return nc


def _get_nc():
    global _cached_nc
    if _cached_nc is None:
        _cached_nc = _build_device_kernel()
    return _cached_nc


def _make_consts():
    ncv = np.zeros(16, np.float32)
    ncv[:N_RBF] = (np.arange(1, N_RBF + 1) * np.pi).astype(np.float32)
    return np.broadcast_to(ncv, (P, 16)).copy()


def _run_device(xyz, trace=False, tmpdir=None):
    from concourse import bass_utils

    nc = _get_nc()
    consts = _make_consts()
    ext = np.concatenate([xyz[-HALF:], xyz, xyz[:HALF]], axis=0)
    in_maps = []
    for c in range(N_CORES):
        base = c * J_PER_CORE
        winc = np.ascontiguousarray(ext[base : base + WIN_ROWS])
        in_maps.append({"win": winc, "consts": consts})
    kwargs = {}
    if trace:
        kwargs = dict(trace=True)
        if tmpdir is not None:
            kwargs["tmpdir"] = tmpdir
    res = bass_utils.run_bass_kernel_spmd(
        nc, in_maps, core_ids=list(range(N_CORES)), **kwargs
    )
    shards = [res.results[c]["out"] for c in range(N_CORES)]
    full = np.concatenate(shards, axis=0).astype(np.float32)
    return full, res


def kernel(xyz, nbr_list, angle_list):
    xyz = np.asarray(xyz, dtype=np.float32)
    if not _graph_matches(nbr_list, angle_list):
        return _fallback_numpy(xyz, nbr_list, angle_list)
    out, _ = _run_device(xyz)
    return out


# revision 19
# speedup vs baseline: 1.1635x; 1.0340x over previous
"""Bass/Trainium2 kernel for nn_DimeNet_22737556865501 (optimized v2).

Same math as the baseline (circulant-graph collapse to dense per-atom work)
with these performance changes:

1. delta-symmetry: alpha[a,b] is symmetric, so the pair chain (ab, amg, den,
   ratio, arctan) is computed only for the 8 circular shifts delta=1..8
   ([K, delta, a] layout, a innermost/unit-stride) instead of all 256 (a,b)
   pairs; rows delta=9..15 of the contraction tensor are cheap copies
   (alpha[d', a] = alpha[16-d', (a+d') % 16]).
2. ratio = sqrt(amg/den) is computed as amg * rsqrt(|amg*den|) using the
   single ACT function Abs_reciprocal_sqrt, so pass 1 needs ONE activation
   table set (abs_reciprocal_sqrt_and_small) and one transcendental op per
   pair instead of five (Square/Ln/Square/Ln/Exp).  1/d and d also come from
   the same function.  Clamps (max(amg,0), max(prod,1e-30)) make it NaN-free.
3. bf16 + DVE 2x mode for everything after the cancellation-sensitive
   subtract amg = ab - G (which stays f32 internally, bf16 output): the
   clamp/prod/ratio chain, the full contraction (products + binary tree),
   all with unit-stride innermost APs.  Shifted reads (a+delta) are split
   into even-delta (4B-aligned -> 2x) and odd-delta instructions.
4. Engine rebalance: GPSIMD runs the odd-delta contraction products and the
   sin-argument ops; ACT runs the transcendentals, the dc-power squares and
   scaling; DVE the rest.

Sharding: unchanged — atoms partitioned across 8 cores, no collective.
"""

import numpy as np

N_ATOMS = 32768
DEG = 16
HALF = DEG // 2
N_CORES = 8
J_PER_CORE = N_ATOMS // N_CORES  # 4096
P = 128
N_TILES = J_PER_CORE // P  # 32
K_BATCH = 8
N_SUPER = N_TILES // K_BATCH  # 4
WIN_ROWS = J_PER_CORE + DEG  # 4112
N_RBF = 6
CUTOFF = 5.0
ENV_P = 6
A_ = -(ENV_P + 1) * (ENV_P + 2) / 2.0  # -28
B_ = float(ENV_P * (ENV_P + 2))  # 48
C_ = -ENV_P * (ENV_P + 1) / 2.0  # -21
TWO_PI = float(2.0 * np.pi)
INV_2PI = float(1.0 / (2.0 * np.pi))

OFFS = np.concatenate([np.arange(1, HALF + 1), -np.arange(1, HALF + 1)])

_cached_nc = None


def _expected_graph():
    half = HALF
    offs = np.concatenate([np.arange(1, half + 1), N_ATOMS - np.arange(1, half + 1)])
    j = np.arange(N_ATOMS)
    nbr_dst = (j[:, None] + offs[None, :]) % N_ATOMS
    nbr_list = np.stack([np.repeat(j, DEG), nbr_dst.reshape(-1)], 1)
    o1, o2 = np.meshgrid(offs, offs, indexing="ij")
    keep = o1 != o2
    o1, o2 = o1[keep], o2[keep]
    i = (j[:, None] + o1[None, :]) % N_ATOMS
    k = (j[:, None] + o2[None, :]) % N_ATOMS
    jc = np.broadcast_to(j[:, None], i.shape)
    angle_list = np.stack([i.reshape(-1), jc.reshape(-1), k.reshape(-1)], 1)
    return nbr_list.astype(np.int64), angle_list.astype(np.int64)


def _graph_matches(nbr_list, angle_list):
    if nbr_list.shape != (N_ATOMS * DEG, 2):
        return False
    if angle_list.shape != (N_ATOMS * DEG * (DEG - 1), 3):
        return False
    exp_nbr, exp_ang = _expected_graph()
    return np.array_equal(np.asarray(nbr_list), exp_nbr) and np.array_equal(
        np.asarray(angle_list), exp_ang
    )


def _fallback_numpy(xyz, nbr_list, angle_list):
    """Exact numpy mirror of the jax reference (general graph)."""
    xyz = np.asarray(xyz, dtype=np.float32)
    nbr = np.asarray(nbr_list)
    ang = np.asarray(angle_list)
    E = nbr.shape[0]
    r_ji = xyz[ang[:, 0]] - xyz[ang[:, 1]]
    r_jk = xyz[ang[:, 2]] - xyz[ang[:, 1]]
    dot = np.sum(r_ji * r_jk, axis=-1)
    crs = np.linalg.norm(np.cross(r_ji, r_jk), axis=-1)
    alpha = np.arctan2(crs, dot)
    diff = xyz[nbr[:, 0]] - xyz[nbr[:, 1]]
    d = np.linalg.norm(diff, axis=-1)
    n = np.arange(1, N_RBF + 1, dtype=xyz.dtype)
    dc = (d / CUTOFF)[:, None]
    env = 1.0 / dc + A_ * dc ** (ENV_P - 1) + B_ * dc**ENV_P + C_ * dc ** (ENV_P + 1)
    e_rbf = env * np.sin(n * np.pi * dc)
    keys = nbr[:, 0] * N_ATOMS + nbr[:, 1]
    order = np.argsort(keys, kind="stable")
    ji_idx = order[np.searchsorted(keys[order], ang[:, 1] * N_ATOMS + ang[:, 0])]
    kj_idx = order[np.searchsorted(keys[order], ang[:, 2] * N_ATOMS + ang[:, 1])]
    trip = alpha[:, None] * e_rbf[kj_idx]
    out = np.zeros((E, N_RBF), dtype=np.float32)
    np.add.at(out, ji_idx, trip.astype(np.float32))
    return out


# ---------------------------------------------------------------------------
# Device kernel
# ---------------------------------------------------------------------------


def _build_device_kernel():
    import concourse.bacc as bacc
    import concourse.mybir as mybir
    from concourse.bass_types import AP
    from concourse.tile import TileContext

    F32 = mybir.dt.float32
    BF16 = mybir.dt.bfloat16
    I32 = mybir.dt.int32
    ALU = mybir.AluOpType
    ACT = mybir.ActivationFunctionType

    # Steer activation-table loading: pass 1 resolves only to
    # abs_reciprocal_sqrt_and_small, pass 2 only to trig_and_small, so the
    # whole kernel performs exactly two table loads.
    from concourse.hw_specs import get_activation_tables

    tabs = get_activation_tables("gen3")
    for name, fns in tabs.items():
        if name != "abs_reciprocal_sqrt_and_small":
            fns.discard(ACT.Abs_reciprocal_sqrt)
        if name != "trig_and_small":
            fns.discard(ACT.Sin)
            fns.discard(ACT.Arctan)
        if name not in ("abs_reciprocal_sqrt_and_small", "trig_and_small"):
            fns.discard(ACT.Copy)
            fns.discard(ACT.Identity)
            fns.discard(ACT.Square)

    def sub(base: AP, off: int, dims) -> AP:
        return AP(
            tensor=base.tensor,
            offset=base.offset + off,
            ap=[list(base.ap[0]), *[list(d) for d in dims]],
        )

    nc = bacc.Bacc("TRN2", target_bir_lowering=False, debug=False, num_devices=N_CORES)
    win = nc.dram_tensor("win", [WIN_ROWS, 3], F32, kind="ExternalInput")
    consts = nc.dram_tensor("consts", [P, 16], F32, kind="ExternalInput")
    out = nc.dram_tensor("out", [J_PER_CORE * DEG, N_RBF], F32, kind="ExternalOutput")

    K = K_BATCH
    with TileContext(nc) as tc:
        with (
            tc.tile_pool(name="cst", bufs=1) as cpool,
            tc.tile_pool(name="carry", bufs=1) as carry,
            tc.tile_pool(name="p1", bufs=2) as p1,
            tc.tile_pool(name="p2", bufs=1) as p2,
        ):
            nco = cpool.tile([P, 16], F32)
            nc.sync.dma_start(nco[:], consts[:])

            ratio_t = [carry.tile([P, K * 128], BF16, name=f"ratio{s}") for s in range(N_SUPER)]
            sred_t = [carry.tile([P, K * 96], F32, name=f"sred{s}") for s in range(N_SUPER)]
            env_t = [carry.tile([P, K * 16], BF16, name=f"env{s}") for s in range(N_SUPER)]

            # ---------------- pass 1: geometry + arsqrt ----------------
            for s in range(N_SUPER):
                w = p1.tile([P, K * 51], F32, tag="w")
                vt = p1.tile([P, K * 72], F32, tag="vt")  # [K][c3][a24]
                g = p1.tile([P, K * 144], F32, tag="g")  # [K][d9][a16]
                gtmp = p1.tile([P, K * 144], F32, tag="gtmp")
                gtmp2 = p1.tile([P, K * 144], F32, tag="gtmp2")
                d24 = p1.tile([P, K * 24], F32, tag="d24")
                arsq0 = p1.tile([P, K * 16], F32, tag="arsq0")
                ab = p1.tile([P, K * 128], F32, tag="ab")  # [K][d8][a16]
                amg = p1.tile([P, K * 128], BF16, tag="amg")
                den = p1.tile([P, K * 128], BF16, tag="den")
                amgc = p1.tile([P, K * 128], BF16, tag="amgc")
                prod = p1.tile([P, K * 128], BF16, tag="prod")
                prodc = p1.tile([P, K * 128], BF16, tag="prodc")
                r2 = p1.tile([P, K * 128], BF16, tag="r2")
                dcp = p1.tile([P, K * 16], F32, tag="dcp")
                inv2 = p1.tile([P, K * 16], F32, tag="inv2")
                dc2 = p1.tile([P, K * 16], F32, tag="dc2")
                dc4 = p1.tile([P, K * 16], F32, tag="dc4")
                x5 = p1.tile([P, K * 16], F32, tag="x5")
                h1 = p1.tile([P, K * 16], F32, tag="h1")
                h2 = p1.tile([P, K * 16], F32, tag="h2")
                h3 = p1.tile([P, K * 16], F32, tag="h3")
                sa2 = p1.tile([P, K * 96], F32, tag="sa2")  # [K][n6][b16]
                st2 = p1.tile([P, K * 96], F32, tag="st2")
                ki = p1.tile([P, K * 96], I32, tag="ki")
                kf = p1.tile([P, K * 96], F32, tag="kf")

                src = AP(tensor=win, offset=s * K * P * 3, ap=[[3, P], [P * 3, K], [1, 51]])
                nc.sync.dma_start(w[:], src)
                wa, va = w[:], vt[:]

                # V_T[K][c][a]: a=0..7 <- +1..+8 (win rows +9..+16), a=8..15 <- -1..-8
                nc.vector.tensor_tensor(
                    sub(va, 0, [[72, K], [24, 3], [1, 8]]),
                    sub(wa, 27, [[51, K], [1, 3], [3, 8]]),
                    sub(wa, 24, [[51, K], [1, 3], [0, 8]]),
                    ALU.subtract,
                )
                nc.vector.tensor_tensor(
                    sub(va, 8, [[72, K], [24, 3], [1, 8]]),
                    sub(wa, 21, [[51, K], [1, 3], [-3, 8]]),
                    sub(wa, 24, [[51, K], [1, 3], [0, 8]]),
                    ALU.subtract,
                )
                # circular halo: V_T[c][16+y] = V_T[c][y]
                nc.vector.tensor_copy(
                    sub(va, 16, [[72, K], [24, 3], [1, 8]]),
                    sub(va, 0, [[72, K], [24, 3], [1, 8]]),
                )

                # Gram, delta-packed: G[d][a] = sum_c V[a+d]c * V[a]c, d=0..8
                ga = g[:]
                nc.gpsimd.tensor_tensor(
                    sub(ga, 0, [[144, K], [16, 9], [1, 16]]),
                    sub(va, 0, [[72, K], [1, 9], [1, 16]]),
                    sub(va, 0, [[72, K], [0, 9], [1, 16]]),
                    ALU.mult,
                )
                for c, gt in ((1, gtmp), (2, gtmp2)):
                    nc.gpsimd.tensor_tensor(
                        sub(gt[:], 0, [[144, K], [16, 9], [1, 16]]),
                        sub(va, 24 * c, [[72, K], [1, 9], [1, 16]]),
                        sub(va, 24 * c, [[72, K], [0, 9], [1, 16]]),
                        ALU.mult,
                    )
                    nc.vector.tensor_tensor(
                        sub(ga, 0, [[144, K], [1, 144]]),
                        sub(ga, 0, [[144, K], [1, 144]]),
                        sub(gt[:], 0, [[144, K], [1, 144]]),
                        ALU.add,
                    )

                # 1/d and d from row delta=0 (n2), plus circular halo of d
                nc.scalar.activation(
                    sub(arsq0[:], 0, [[16, K], [1, 16]]),
                    sub(ga, 0, [[144, K], [1, 16]]),
                    ACT.Abs_reciprocal_sqrt,
                )
                nc.vector.tensor_tensor(
                    sub(d24[:], 0, [[24, K], [1, 16]]),
                    sub(ga, 0, [[144, K], [1, 16]]),
                    sub(arsq0[:], 0, [[16, K], [1, 16]]),
                    ALU.mult,
                )
                nc.vector.tensor_copy(
                    sub(d24[:], 16, [[24, K], [1, 8]]),
                    sub(d24[:], 0, [[24, K], [1, 8]]),
                )

                # ab[d][a] = d[a] * d[a+d], d=1..8 (odd/even split by alignment)
                nc.vector.tensor_tensor(
                    sub(ab[:], 0, [[128, K], [32, 4], [1, 16]]),
                    sub(d24[:], 1, [[24, K], [2, 4], [1, 16]]),
                    sub(d24[:], 0, [[24, K], [0, 4], [1, 16]]),
                    ALU.mult,
                )
                nc.vector.tensor_tensor(
                    sub(ab[:], 16, [[128, K], [32, 4], [1, 16]]),
                    sub(d24[:], 2, [[24, K], [2, 4], [1, 16]]),
                    sub(d24[:], 0, [[24, K], [0, 4], [1, 16]]),
                    ALU.mult,
                )

                # amg/den (f32 compute, bf16 store), clamp, prod, ratio
                gp = sub(ga, 16, [[144, K], [16, 8], [1, 16]])
                abf = sub(ab[:], 0, [[128, K], [16, 8], [1, 16]])
                nc.vector.tensor_tensor(
                    sub(amg[:], 0, [[128, K], [16, 8], [1, 16]]), abf, gp, ALU.subtract
                )
                nc.vector.tensor_tensor(
                    sub(den[:], 0, [[128, K], [16, 8], [1, 16]]), abf, gp, ALU.add
                )
                nc.vector.tensor_scalar_max(amgc[:], amg[:], 0.0)
                nc.vector.tensor_tensor(prod[:], amgc[:], den[:], ALU.mult)
                nc.vector.tensor_scalar_max(prodc[:], prod[:], 1e-30)
                nc.scalar.activation(r2[:], prodc[:], ACT.Abs_reciprocal_sqrt)
                nc.vector.tensor_tensor(ratio_t[s][:], amgc[:], r2[:], ALU.mult)

                # envelope: env2 = 2C'/d*? -> inv2 + dc^5*(2A + dc*(2B + dc*2C'))
                nc.scalar.mul(
                    sub(dcp[:], 0, [[16, K], [1, 16]]),
                    sub(d24[:], 0, [[24, K], [1, 16]]),
                    1.0 / CUTOFF,
                )
                nc.scalar.mul(inv2[:], arsq0[:], 2.0 * CUTOFF)
                nc.vector.tensor_tensor(dc2[:], dcp[:], dcp[:], ALU.mult)
                nc.vector.tensor_tensor(dc4[:], dc2[:], dc2[:], ALU.mult)
                nc.vector.tensor_tensor(x5[:], dc4[:], dcp[:], ALU.mult)
                nc.vector.tensor_scalar(h1[:], dcp[:], 2.0 * C_, 2.0 * B_, ALU.mult, ALU.add)
                nc.vector.tensor_tensor(h2[:], h1[:], dcp[:], ALU.mult)
                nc.vector.scalar_tensor_tensor(h3[:], h2[:], 2.0 * A_, x5[:], ALU.add, ALU.mult)
                nc.vector.tensor_tensor(env_t[s][:], h3[:], inv2[:], ALU.add)

                # sin args [K][n6][b16], range-reduced to [-pi, pi]
                nc.gpsimd.tensor_tensor(
                    sub(sa2[:], 0, [[96, K], [16, 6], [1, 16]]),
                    sub(dcp[:], 0, [[16, K], [0, 6], [1, 16]]),
                    sub(nco[:], 0, [[0, K], [1, 6], [0, 16]]),
                    ALU.mult,
                )
                nc.scalar.mul(st2[:], sa2[:], INV_2PI)
                nc.vector.tensor_copy(ki[:], st2[:])  # round-to-nearest
                nc.vector.tensor_copy(kf[:], ki[:])
                nc.vector.scalar_tensor_tensor(
                    sred_t[s][:], kf[:], -TWO_PI, sa2[:], ALU.mult, ALU.add
                )

            # ---------------- pass 2: trig + contraction ----------------
            tc.no_sync_barrier()
            for s in range(N_SUPER):
                aext = p2.tile([P, K * 256], BF16, tag="aext")  # [K][d16][a16]
                sinv = p2.tile([P, K * 96], BF16, tag="sinv")  # [K][r6][b16]
                erbf = p2.tile([P, K * 192], BF16, tag="erbf")  # [K][r6][b32]
                v16 = p2.tile([P, K * 1536], BF16, tag="v16")  # [K][r6][d16][a16]
                ot = p2.tile([P, K * 96], F32, tag="ot")  # [K][a16][r6]

                ax = aext[:]
                nc.scalar.activation(
                    sub(ax, 16, [[256, K], [1, 128]]),
                    sub(ratio_t[s][:], 0, [[128, K], [1, 128]]),
                    ACT.Arctan,
                )
                nc.gpsimd.memset(sub(ax, 0, [[256, K], [1, 16]]), 0.0)
                # duplicate rows 9..15: aext[d'][a] = aext[16-d'][(a+d')%16]
                for dp in range(9, 16):
                    sb = (16 - dp) * 16
                    nc.scalar.copy(
                        sub(ax, dp * 16, [[256, K], [1, 16 - dp]]),
                        sub(ax, sb + dp, [[256, K], [1, 16 - dp]]),
                    )
                    nc.scalar.copy(
                        sub(ax, dp * 16 + (16 - dp), [[256, K], [1, dp]]),
                        sub(ax, sb, [[256, K], [1, dp]]),
                    )

                nc.scalar.activation(sinv[:], sred_t[s][:], ACT.Sin)
                # erbf[r][b] = env2[b] * sin_r[b], plus circular halo b=16..31
                eb = erbf[:]
                nc.vector.tensor_tensor(
                    sub(eb, 0, [[192, K], [32, 6], [1, 16]]),
                    sub(sinv[:], 0, [[96, K], [16, 6], [1, 16]]),
                    sub(env_t[s][:], 0, [[16, K], [0, 6], [1, 16]]),
                    ALU.mult,
                )
                nc.vector.tensor_copy(
                    sub(eb, 16, [[192, K], [32, 6], [1, 16]]),
                    sub(eb, 0, [[192, K], [32, 6], [1, 16]]),
                )

                # products v16[r][d][a] = aext[d][a] * erbf[r][a+d]
                # out/in0 are flat contiguous 256-runs; in1 is a sliding
                # 16-window (d and a both stride 1)
                vv = v16[:]
                for r in range(N_RBF):
                    nc.vector.tensor_tensor(
                        sub(vv, r * 256, [[1536, K], [32, 8], [1, 16]]),
                        sub(ax, 0, [[256, K], [32, 8], [1, 16]]),
                        sub(eb, r * 32, [[192, K], [2, 8], [1, 16]]),
                        ALU.mult,
                    )
                    nc.gpsimd.tensor_tensor(
                        sub(vv, r * 256 + 16, [[1536, K], [32, 8], [1, 16]]),
                        sub(ax, 16, [[256, K], [32, 8], [1, 16]]),
                        sub(eb, r * 32 + 1, [[192, K], [2, 8], [1, 16]]),
                        ALU.mult,
                    )
                # binary tree over d (in place), then f32 transpose-store to ot
                for half in (128, 64, 32):
                    nc.vector.tensor_tensor(
                        sub(vv, 0, [[1536, K], [256, 6], [1, half]]),
                        sub(vv, 0, [[1536, K], [256, 6], [1, half]]),
                        sub(vv, half, [[1536, K], [256, 6], [1, half]]),
                        ALU.add,
                    )
                nc.vector.tensor_tensor(
                    sub(ot[:], 0, [[96, K], [1, 6], [6, 16]]),
                    sub(vv, 0, [[1536, K], [256, 6], [1, 16]]),
                    sub(vv, 16, [[1536, K], [256, 6], [1, 16]]),
                    ALU.add,
                )
                dst = AP(
                    tensor=out,
                    offset=s * K * P * 96,
                    ap=[[96, P], [96 * P, K], [1, 96]],
                )
                nc.sync.dma_start(dst, ot[:])

    nc.compile()
    return nc


def _get_nc():
    global _cached_nc
    if _cached_nc is None:
        _cached_nc = _build_device_kernel()
    return _cached_nc


def _make_consts():
    ncv = np.zeros(16, np.float32)
    ncv[:N_RBF] = (np.arange(1, N_RBF + 1) * np.pi).astype(np.float32)
    return np.broadcast_to(ncv, (P, 16)).copy()


def _run_device(xyz, trace=False, tmpdir=None):
    from concourse import bass_utils

    nc = _get_nc()
    consts = _make_consts()
    ext = np.concatenate([xyz[-HALF:], xyz, xyz[:HALF]], axis=0)
    in_maps = []
    for c in range(N_CORES):
        base = c * J_PER_CORE
        winc = np.ascontiguousarray(ext[base : base + WIN_ROWS])
        in_maps.append({"win": winc, "consts": consts})
    kwargs = {}
    if trace:
        kwargs = dict(trace=True)
        if tmpdir is not None:
            kwargs["tmpdir"] = tmpdir
    res = bass_utils.run_bass_kernel_spmd(
        nc, in_maps, core_ids=list(range(N_CORES)), **kwargs
    )
    shards = [res.results[c]["out"] for c in range(N_CORES)]
    full = np.concatenate(shards, axis=0).astype(np.float32)
    return full, res


def kernel(xyz, nbr_list, angle_list):
    xyz = np.asarray(xyz, dtype=np.float32)
    if not _graph_matches(nbr_list, angle_list):
        return _fallback_numpy(xyz, nbr_list, angle_list)
    out, _ = _run_device(xyz)
    return out
